# revision 1
# baseline (speedup 1.0000x reference)
"""Trainium2 Bass kernel for nn_GAT_77704548319854.

Math: every per-edge quantity in the reference depends only on the edge's
source node (rows = ent_embed[src], then row-wise ops / linear layers), so the
attention-weighted segment mean collapses exactly:
    h_ent[n] = (cnt[n] * e[n] * c[n]) / (cnt[n] * e[n]) = c[n]   if cnt[n] > 0
    h_ent[n] = 0                                                  if cnt[n] == 0
with c = clip_rownorm(ent_embed) @ W_a.T + b_a. So
    out[n] = relu(c[n]) * 1{n appears in triplets[:,0]}.

Device plan (8 cores, SPMD):
  - edges sharded 8x for the membership mask; nodes sharded 8x for the dense
    compute; cross-core combine via ReduceScatter(add) of the mask.
  - mask built with dma_scatter_add (int16 indices, 256B row stride): 4 range
    passes of 25088 nodes; out-of-range ids clamp to a dump row via an
    unsigned-min trick. Duplicate-index add races are safe for a mask
    (value stays > 0).
  - dense part: rownorm clip, PE transpose + matmul (bias folded in via a
    ones row), relu with the mask as the per-partition activation scale.
"""
import sys

sys.path.insert(0, "/opt/trn_rl_repo")

import numpy as np

import concourse.bacc as bacc
import concourse.bass as bass
import concourse.mybir as mybir
import concourse.tile as tile
from concourse.bass_utils import run_bass_kernel_spmd
from concourse.masks import make_identity

F = 64          # in_dim == out_dim == 64
N_CORES = 8


class Cfg:
    def __init__(self, nodes_per_core, edges_per_core_pad, n_passes, pass_rows,
                 jl=2048, chunk=6272):
        chunk = min(chunk, edges_per_core_pad)
        assert nodes_per_core % 128 == 0
        assert edges_per_core_pad % 128 == 0 and edges_per_core_pad % 16 == 0
        assert pass_rows % 128 == 0
        # scatter row index space must fit int16
        assert 2 * jl + pass_rows + 4 < 32768
        assert n_passes * pass_rows >= N_CORES * nodes_per_core
        assert pass_rows % nodes_per_core == 0
        assert edges_per_core_pad % chunk == 0
        assert chunk % 128 == 0 and chunk % 16 == 0 and chunk <= 7168
        self.npc = nodes_per_core            # nodes per core
        self.ecp = edges_per_core_pad        # padded edges per core
        self.n_passes = n_passes
        self.pass_rows = pass_rows           # mask rows per pass (valid range)
        self.jl = jl                         # jittered dump region length
        self.chunk = chunk                   # idxs per scatter instruction
        self.tiles = nodes_per_core // 128   # [128,64] node tiles per core
        self.np_total = N_CORES * nodes_per_core


# full-size config: N=100000 -> 100352 padded nodes, E=1600000 -> 200704/core
FULL = Cfg(nodes_per_core=12544, edges_per_core_pad=200704, n_passes=4,
           pass_rows=25088)

f32 = mybir.dt.float32
i32 = mybir.dt.int32
u32 = mybir.dt.uint32
i16 = mybir.dt.int16


def build(cfg: Cfg, n_cores=N_CORES, debug_taps=False, scatter_reps=1):
    npc, ecp, tiles = cfg.npc, cfg.ecp, cfg.tiles
    n_passes, pr, jl = cfg.n_passes, cfg.pass_rows, cfg.jl
    # maskbuf layout per pass: rows [0, jl) = below-range dump (jittered),
    # rows [jl, jl+pr) = nodes [base, base+pr), rows [jl+pr, jl+pr+jl) =
    # above-range dump (jittered)
    mb_rows = 2 * jl + pr + 4
    ids_cols = ecp // 16

    n_rounds = 2 if ecp > 2048 * 16 else 1   # halve SBUF working set
    assert ecp % n_rounds == 0
    cols_h = ids_cols // n_rounds            # idx cols per round

    nc = bacc.Bacc("TRN2", target_bir_lowering=False, debug=False,
                   num_devices=n_cores)
    ids_d = nc.dram_tensor("ids", [n_rounds, 16, cols_h], i32,
                           kind="ExternalInput")
    jlo_d = nc.dram_tensor("jlo", [16, cols_h], f32, kind="ExternalInput")
    emb_d = nc.dram_tensor("emb", [npc, F], f32, kind="ExternalInput")
    wa_d = nc.dram_tensor("wa", [F, F], f32, kind="ExternalInput")
    ba_d = nc.dram_tensor("ba", [F], f32, kind="ExternalInput")
    out_d = nc.dram_tensor("out", [npc, F], f32, kind="ExternalOutput")
    if debug_taps:
        dbg_mask = nc.dram_tensor("dbg_mask", [cfg.np_total], f32,
                                  kind="ExternalOutput")
        dbg_rs = nc.dram_tensor("dbg_rs", [npc], f32, kind="ExternalOutput")

    with tile.TileContext(nc) as tc:
        with tc.tile_pool(name="sb", bufs=1) as sb, \
             tc.tile_pool(name="sbt", bufs=3) as sbt, \
             tc.tile_pool(name="ps", bufs=2, space="PSUM") as ps, \
             tc.tile_pool(name="dram", bufs=1, space="DRAM") as dram:

            # ======== phase A: membership mask via scatter-add ========
            maskbufs = [dram.tile([mb_rows, F], f32, name=f"mb{r}")
                        for r in range(n_passes)]

            # zero-fill the node-row region of each maskbuf (the dump regions
            # are never read; scatter accumulates so node col 0 must start 0)
            zchunk = 128 * 2048
            zt = sb.tile([128, 2048], f32)
            nc.vector.memset(zt[:], 0.0)
            for r in range(n_passes):
                total = pr * F                    # pr*64 is a multiple of 128
                flat = maskbufs[r][jl:jl + pr, :].rearrange("r f -> (r f)")
                off = 0
                while off < total:
                    n = min(zchunk, total - off)
                    assert n % 128 == 0
                    nc.sync.dma_start(
                        out=flat[off:off + n].rearrange("(p x) -> p x", p=128),
                        in_=zt[:, :n // 128])
                    off += n

            # jlo = per-edge jittered below-dump row in [0, jl), replicated to
            # partition groups 0 and 1 (the Q7 tx/rx pair reads both)
            jlo = sb.tile([32, cols_h], f32)
            for g in range(2):
                nc.sync.dma_start(out=jlo[16 * g:16 * (g + 1), :],
                                  in_=jlo_d[:])
            ones = sb.tile([128, cfg.chunk // 128], f32)
            nc.vector.memset(ones[:], 1.0)

            # mask row for pass r: row = jlo + clamp(xm - (r*pr - jl), 0, C)
            # where xm = id - jlo and C = jl + pr. In-range ids map to
            # jl + (id - base); out-of-range ids land in a jittered dump row.
            C = float(jl + pr)
            idsf = sb.tile([32, cols_h], f32)
            dsubf = sb.tile([32, cols_h], f32)
            idx16s = []
            for b in range(2):
                idx16 = sbt.tile([128, cols_h], i16, tag="idx16", bufs=2,
                                 name=f"idx16_{b}")
                for pstart in (32, 64, 96):
                    # partitions 32+ are never read by queue 0's Q7 pair;
                    # zeroed once to satisfy the simulator's bounds check
                    nc.vector.memset(idx16[pstart:pstart + 32, :], 0)
                idx16s.append(idx16)

            per_round_chunks = (ecp // n_rounds) // cfg.chunk
            cpc = cfg.chunk // 16             # idx cols per chunk
            for h in range(n_rounds):
                # edge ids as exact f32 (SWDGE casts i32->f32 during the DMA)
                for g in range(2):
                    nc.gpsimd.dma_start(out=idsf[16 * g:16 * (g + 1), :],
                                        in_=ids_d[h])
                # xm = id - jlo (in place)
                nc.vector.tensor_tensor(out=idsf[:], in0=idsf[:], in1=jlo[:],
                                        op=mybir.AluOpType.subtract)
                for r in range(n_passes):
                    nc.vector.tensor_scalar(
                        out=dsubf[:], in0=idsf[:],
                        scalar1=float(r * pr - jl), scalar2=0.0,
                        op0=mybir.AluOpType.subtract,
                        op1=mybir.AluOpType.max)
                    nc.vector.tensor_scalar_min(out=dsubf[:], in0=dsubf[:],
                                                scalar1=C)
                    nc.vector.tensor_tensor(out=dsubf[:], in0=dsubf[:],
                                            in1=jlo[:],
                                            op=mybir.AluOpType.add)
                    idx16 = idx16s[(h * n_passes + r) % 2]
                    nc.vector.tensor_copy(out=idx16[0:32, :], in_=dsubf[:])
                    for _rep in range(scatter_reps):
                        for c in range(per_round_chunks):
                            # rotate over 4 columns so consecutive chunks
                            # carry no WAW dependency (Tile tracks AP ranges)
                            col = c % 4
                            nc.gpsimd.dma_scatter_add(
                                maskbufs[r][:, col:col + 1],
                                ones[:][:, :, None],
                                idx16[:, c * cpc:(c + 1) * cpc],
                                cfg.chunk, cfg.chunk, 1, elem_step=F)

            # extract columns 0..3 of the node rows, sum, into the
            # natural-order mask
            mask_c = dram.tile([cfg.np_total], f32)
            for r in range(n_passes):
                mext = sbt.tile([128, (pr // 128) * 4], f32, tag="mext",
                                bufs=2)
                nc.sync.dma_start(
                    out=mext[:].rearrange("p (t f) -> p t f", f=4),
                    in_=maskbufs[r][jl:jl + pr, 0:4]
                        .rearrange("(p t) f -> p t f", p=128))
                msum = sbt.tile([128, pr // 128], f32, tag="msum", bufs=2)
                nc.vector.tensor_reduce(
                    out=msum[:],
                    in_=mext[:].rearrange("p (t f) -> p t f", f=4),
                    axis=mybir.AxisListType.X, op=mybir.AluOpType.add)
                nc.sync.dma_start(
                    out=mask_c[r * pr:(r + 1) * pr]
                        .rearrange("(p t) -> p t", p=128),
                    in_=msum[:])

            # combine across cores; core c receives its contiguous node slice
            rs_out = dram.tile([npc], f32)
            nc.gpsimd.collective_compute(
                "ReduceScatter", mybir.AluOpType.add,
                replica_groups=[list(range(n_cores))],
                ins=[mask_c[:]], outs=[rs_out[:]])

            if debug_taps:
                tmp = sb.tile([128, cfg.np_total // 128], f32)
                nc.sync.dma_start(
                    out=tmp[:],
                    in_=mask_c[:].rearrange("(p t) -> p t", p=128))
                nc.sync.dma_start(
                    out=dbg_mask[:].rearrange("(p t) -> p t", p=128),
                    in_=tmp[:])
                tmp2 = sb.tile([128, tiles], f32)
                nc.sync.dma_start(
                    out=tmp2[:],
                    in_=rs_out[:].rearrange("(p t) -> p t", p=128))
                nc.sync.dma_start(
                    out=dbg_rs[:].rearrange("(p t) -> p t", p=128),
                    in_=tmp2[:])

            mask_sb = sb.tile([128, tiles], f32)
            nc.sync.dma_start(out=mask_sb[:],
                              in_=rs_out[:].rearrange("(p t) -> p t", p=128))
            nc.vector.tensor_scalar_min(out=mask_sb[:], in0=mask_sb[:],
                                        scalar1=1.0)

            # ======== phase B: dense per-node compute ========
            emb_sb = sb.tile([128, tiles * F], f32)
            nc.sync.dma_start(out=emb_sb[:],
                              in_=emb_d[:].rearrange("(p t) f -> p (t f)",
                                                     p=128))
            emb3 = emb_sb[:].rearrange("p (t f) -> p t f", f=F)
            # out_sb doubles as the squared-embedding scratch early on
            out_sb = sb.tile([128, tiles * F], f32)
            sq = out_sb
            nc.vector.tensor_mul(out=sq[:], in0=emb_sb[:], in1=emb_sb[:])
            ssq = sb.tile([128, tiles], f32)
            nc.vector.tensor_reduce(out=ssq[:],
                                    in_=sq[:].rearrange("p (t f) -> p t f",
                                                        f=F),
                                    axis=mybir.AxisListType.X,
                                    op=mybir.AluOpType.add)
            nrm = sb.tile([128, tiles], f32)
            nc.scalar.sqrt(out=nrm[:], in_=ssq[:])
            nc.vector.tensor_scalar_add(out=nrm[:], in0=nrm[:], scalar1=1e-7)
            rec = sb.tile([128, tiles], f32)
            nc.vector.reciprocal(out=rec[:], in_=nrm[:])
            nc.vector.tensor_scalar_min(out=rec[:], in0=rec[:], scalar1=1.0)
            # h = emb * scale (broadcast scale along features)
            nc.vector.tensor_tensor(
                out=emb3, in0=emb3,
                in1=rec[:][:, :, None].to_broadcast([128, tiles, F]),
                op=mybir.AluOpType.mult)

            ident = sb.tile([128, 128], f32)
            make_identity(nc, ident[:])
            # W_ab = [W_a.T ; b_a]  (bias via ones row in lhsT)
            wa_sb = sb.tile([F, F], f32)
            nc.sync.dma_start(out=wa_sb[:], in_=wa_d[:])
            wat_ps = ps.tile([F, F], f32, tag="wat")
            nc.tensor.transpose(out=wat_ps[:], in_=wa_sb[:],
                                identity=ident[:F, :F])
            w_ab = sb.tile([F + 1, F], f32)
            nc.vector.tensor_copy(out=w_ab[0:F, :], in_=wat_ps[:])
            nc.sync.dma_start(out=w_ab[F:F + 1, :], in_=ba_d[None, :])

            relu = mybir.ActivationFunctionType.Relu
            for t in range(tiles):
                h_t = emb_sb[:, t * F:(t + 1) * F]
                ht_ps = ps.tile([F, 128], f32, tag="ht")
                nc.tensor.transpose(out=ht_ps[:], in_=h_t, identity=ident[:])
                ht_sb = sbt.tile([F + 1, 128], f32, tag="hts")
                nc.vector.tensor_copy(out=ht_sb[0:F, :], in_=ht_ps[:])
                nc.vector.memset(ht_sb[F:F + 1, :], 1.0)
                c_ps = ps.tile([128, F], f32, tag="cps")
                nc.tensor.matmul(c_ps[:], ht_sb[:], w_ab[:],
                                 start=True, stop=True)
                nc.scalar.activation(out=out_sb[:, t * F:(t + 1) * F],
                                     in_=c_ps[:], func=relu,
                                     scale=mask_sb[:, t:t + 1])

            nc.sync.dma_start(
                out=out_d[:].rearrange("(p t) f -> p (t f)", p=128),
                in_=out_sb[:])

    nc.compile()
    return nc


_cache = {}


def _get_nc():
    if "nc" not in _cache:
        _cache["nc"] = build(FULL)
    return _cache["nc"]


def _in_maps(cfg: Cfg, triplets, ent_embed, W_a, b_a):
    src = np.ascontiguousarray(triplets[:, 0]).astype(np.int32)
    e_total = src.shape[0]
    epc = e_total // N_CORES
    n = ent_embed.shape[0]
    emb_pad = np.zeros((cfg.np_total, F), np.float32)
    emb_pad[:n] = np.asarray(ent_embed, np.float32)
    wa = np.ascontiguousarray(np.asarray(W_a, np.float32))
    ba = np.ascontiguousarray(np.asarray(b_a, np.float32))
    n_rounds = 2 if cfg.ecp > 2048 * 16 else 1
    cols_h = cfg.ecp // 16 // n_rounds
    jlo = ((np.arange(16, dtype=np.int64)[:, None] * 53
            + np.arange(cols_h, dtype=np.int64)[None, :] * 37) % cfg.jl
           ).astype(np.float32)
    maps = []
    for c in range(N_CORES):
        s = src[c * epc:(c + 1) * epc]
        sp = np.empty(cfg.ecp, np.int32)
        sp[:epc] = s
        sp[epc:] = s[0]  # pad with a duplicate edge (idempotent for the mask)
        maps.append({
            "ids": sp.reshape(n_rounds, 16, cols_h),
            "jlo": jlo,
            "emb": emb_pad[c * cfg.npc:(c + 1) * cfg.npc],
            "wa": wa,
            "ba": ba,
        })
    return maps


def kernel(triplets, ent_embed, W_a, b_a, W_a2, b_a2):
    # W_a2 / b_a2 cancel algebraically (see module docstring)
    nc = _get_nc()
    maps = _in_maps(FULL, triplets, ent_embed, W_a, b_a)
    res = run_bass_kernel_spmd(nc, maps, core_ids=list(range(N_CORES)))
    out = np.concatenate([r["out"] for r in res.results], axis=0)
    return np.ascontiguousarray(out[:ent_embed.shape[0]])



# revision 6
# speedup vs baseline: 68.4309x; 68.4309x over previous
"""Trainium2 Bass kernel for nn_GAT_77704548319854.

Math: every per-edge quantity in the reference depends only on the edge's
source node (rows = ent_embed[src], then row-wise ops / linear layers), so the
attention-weighted segment mean collapses exactly:
    h_ent[n] = (cnt[n] * e[n] * c[n]) / (cnt[n] * e[n]) = c[n]   if cnt[n] > 0
    h_ent[n] = 0                                                  if cnt[n] == 0
with c = clip_rownorm(ent_embed) @ W_a.T + b_a. So
    out[n] = relu(c[n]) * 1{n appears in triplets[:,0]}.

Device plan (8 cores, SPMD):
  Phase A (membership): one-hot matmul histogram. Edges sharded 8x; each core
  counts its 200704 edges over the full padded node space 100352 = 128*784.
  Host splits each source id into lo = id//784 (128 bins) and hi = id%784
  (784 bins). Per batch of 128 edges (one per partition), DVE builds two
  fp16 one-hots via tensor_scalar(is_equal) against iota rows (4x mode), and
  PE accumulates counts[lo, hi] += onehot_lo.T @ onehot_hi into two PSUM
  banks (392 cols each). This replaces the previous dma_scatter_add design
  (~80 ns/edge on GPSIMD) with ~2.8 ns/edge on DVE/PE.
  Cross-core combine: ReduceScatter(add) of the flattened counts; each core
  gets its contiguous 12544-node slice. mask = min(count, 1).

  Phase B (dense per-node): host supplies embT = emb.T per core. For each
  128-node tile, matmul lhsT = [embT_tile ; invs_row] (65 x 128) with
  rhs = [W_a.T ; b_a] (65 x 64) gives raw@W + invs_n * b. The row-norm clip
  scale s_n = min(1, 1/(nrm+1e-7)) and invs_n = max(1, nrm+1e-7) satisfy
  s*invs = 1, so activation relu with per-node scale (mask*s) yields
  relu(mask*(s*raw@W + b)) exactly. invs_row is built on device from the
  natural-layout emb (norms), PE-transposed, and DMA-flattened into
  partition 64 of the embT tile.
"""
import sys

sys.path.insert(0, "/opt/trn_rl_repo")

import numpy as np

import concourse.bacc as bacc
import concourse.bass as bass
import concourse.mybir as mybir
import concourse.tile as tile
from concourse.bass_utils import run_bass_kernel_spmd

F = 64             # in_dim == out_dim == 64
N_CORES = 8
LO = 128           # lo bins (id // 784)
HI = 784           # hi bins (id % 784)
NP_TOTAL = LO * HI         # 100352 padded nodes
NPC = NP_TOTAL // N_CORES  # 12544 nodes per core
TILES = NPC // 128         # 98 node tiles per core
E_TOTAL = 1600000
EPC = 200704               # padded edges per core (128 * 1568)
B = EPC // 128             # 1568 batches per core

f32 = mybir.dt.float32
fp16 = mybir.dt.float16
i16 = mybir.dt.int16


def build(n_cores=N_CORES, n_batches=B):
    nc = bacc.Bacc("TRN2", target_bir_lowering=False, debug=False,
                   num_devices=n_cores)
    lo_d = nc.dram_tensor("lo", [128, n_batches], f32, kind="ExternalInput")
    hi_d = nc.dram_tensor("hi", [128, n_batches], f32, kind="ExternalInput")
    embt_d = nc.dram_tensor("embt", [F, NPC], f32, kind="ExternalInput")
    emb_d = nc.dram_tensor("emb", [NPC, F], f32, kind="ExternalInput")
    wab_d = nc.dram_tensor("wab", [F + 1, F], f32, kind="ExternalInput")
    out_d = nc.dram_tensor("out", [NPC, F], f32, kind="ExternalOutput")

    relu = mybir.ActivationFunctionType.Relu
    eq = mybir.AluOpType.is_equal

    with tile.TileContext(nc) as tc:
        with tc.tile_pool(name="sb", bufs=1) as sb, \
             tc.tile_pool(name="sbt", bufs=3) as sbt, \
             tc.tile_pool(name="ps", bufs=1, space="PSUM") as ps, \
             tc.tile_pool(name="dram", bufs=1, space="DRAM") as dram:

            # ---- iota rows for the one-hot compares ----
            ioh_i = sb.tile([128, HI], i16)
            nc.gpsimd.iota(ioh_i[:], pattern=[[1, HI]], base=0,
                           channel_multiplier=0)
            ioh = sb.tile([128, HI], fp16)
            nc.vector.tensor_copy(out=ioh[:], in_=ioh_i[:])
            iol_i = sb.tile([128, LO], i16)
            nc.gpsimd.iota(iol_i[:], pattern=[[1, LO]], base=0,
                           channel_multiplier=0)
            iol = sb.tile([128, LO], fp16)
            nc.vector.tensor_copy(out=iol[:], in_=iol_i[:])

            lo_sb = sb.tile([128, n_batches], f32)
            nc.sync.dma_start(out=lo_sb[:], in_=lo_d[:])
            hi_sb = sb.tile([128, n_batches], f32)
            nc.sync.dma_start(out=hi_sb[:], in_=hi_d[:])

            # ---- phase A: one-hot matmul histogram ----
            ps0 = ps.tile([128, HI // 2], f32, tag="ps0")
            ps1 = ps.tile([128, HI // 2], f32, tag="ps1")
            for b in range(n_batches):
                ohl = sbt.tile([128, LO], fp16, tag="ohl")
                nc.vector.tensor_scalar(
                    out=ohl[:], in0=iol[:], scalar1=lo_sb[:, b:b + 1],
                    scalar2=None, op0=eq)
                ohh = sbt.tile([128, HI], fp16, tag="ohh")
                nc.vector.tensor_scalar(
                    out=ohh[:], in0=ioh[:], scalar1=hi_sb[:, b:b + 1],
                    scalar2=None, op0=eq)
                nc.tensor.matmul(ps0[:], ohl[:], ohh[:, 0:HI // 2],
                                 start=(b == 0), stop=(b == n_batches - 1))
                nc.tensor.matmul(ps1[:], ohl[:], ohh[:, HI // 2:HI],
                                 start=(b == 0), stop=(b == n_batches - 1))

            cnt_sb = sb.tile([128, HI], f32)
            nc.vector.tensor_copy(out=cnt_sb[:, 0:HI // 2], in_=ps0[:])
            nc.vector.tensor_copy(out=cnt_sb[:, HI // 2:HI], in_=ps1[:])
            mask_c = dram.tile([NP_TOTAL], f32)
            nc.sync.dma_start(
                out=mask_c[:].rearrange("(p t) -> p t", p=128),
                in_=cnt_sb[:])
            rs_out = dram.tile([NPC], f32)
            nc.gpsimd.collective_compute(
                "ReduceScatter", mybir.AluOpType.add,
                replica_groups=[list(range(n_cores))],
                ins=[mask_c[:]], outs=[rs_out[:]])

            # ---- phase B prep: embeddings, norms, weights ----
            embt_sb = sb.tile([F + 1, NPC], f32)
            nc.sync.dma_start(out=embt_sb[0:F, :], in_=embt_d[:])
            wab_sb = sb.tile([F + 1, F], f32)
            nc.sync.dma_start(out=wab_sb[:], in_=wab_d[:])
            emb_sb = sb.tile([128, TILES * F], f32)
            nc.sync.dma_start(
                out=emb_sb[:],
                in_=emb_d[:].rearrange("(p t) f -> p (t f)", p=128))

            sq = sb.tile([128, TILES * F], f32)
            nc.vector.tensor_mul(out=sq[:], in0=emb_sb[:], in1=emb_sb[:])
            nrm = sb.tile([128, TILES], f32)
            nc.vector.tensor_reduce(
                out=nrm[:],
                in_=sq[:].rearrange("p (t f) -> p t f", f=F),
                axis=mybir.AxisListType.X, op=mybir.AluOpType.add)
            nc.scalar.sqrt(out=nrm[:], in_=nrm[:])
            nc.vector.tensor_scalar_add(out=nrm[:], in0=nrm[:], scalar1=1e-7)
            s_sb = sb.tile([128, TILES], f32)
            nc.vector.reciprocal(out=s_sb[:], in_=nrm[:])
            nc.vector.tensor_scalar_min(out=s_sb[:], in0=s_sb[:], scalar1=1.0)
            invs = sb.tile([128, TILES], f32)
            nc.vector.tensor_scalar_max(out=invs[:], in0=nrm[:], scalar1=1.0)

            # invs is [128, TILES] in natural node order (node = p*TILES + t),
            # which is exactly embt's column order: flatten through DRAM into
            # partition F of the embT tile (SBUF free-dims can't span
            # partitions, so bounce via DRAM).
            invs_flat = dram.tile([NPC], f32)
            nc.sync.dma_start(
                out=invs_flat[:].rearrange("(p t) -> p t", p=128),
                in_=invs[:])
            nc.sync.dma_start(out=embt_sb[F:F + 1, :],
                              in_=invs_flat[:][None, :])

            # ---- mask and combined activation scale ----
            mask_sb = sb.tile([128, TILES], f32)
            nc.sync.dma_start(
                out=mask_sb[:],
                in_=rs_out[:].rearrange("(p t) -> p t", p=128))
            nc.vector.tensor_scalar_min(out=mask_sb[:], in0=mask_sb[:],
                                        scalar1=1.0)
            a_sb = sb.tile([128, TILES], f32)
            nc.vector.tensor_mul(out=a_sb[:], in0=mask_sb[:], in1=s_sb[:])

            # ---- phase B: per-tile matmul + masked relu ----
            # node tile t gathers columns {p*TILES + t} (natural layout), so
            # the lhsT slice is strided along the free axis.
            embt3 = embt_sb[:].rearrange("k (p t) -> k p t", t=TILES)
            out_sb = sb.tile([128, TILES * F], f32)
            for t in range(TILES):
                psb = ps.tile([128, F], f32, tag="psb", bufs=2)
                nc.tensor.matmul(psb[:],
                                 embt3[:, :, t],
                                 wab_sb[:], start=True, stop=True)
                nc.scalar.activation(out=out_sb[:, t * F:(t + 1) * F],
                                     in_=psb[:], func=relu,
                                     scale=a_sb[:, t:t + 1])

            nc.sync.dma_start(
                out=out_d[:].rearrange("(p t) f -> p (t f)", p=128),
                in_=out_sb[:])

    nc.compile()
    return nc


_cache = {}


def _get_nc():
    if "nc" not in _cache:
        _cache["nc"] = build()
    return _cache["nc"]


def _in_maps(triplets, ent_embed, W_a, b_a):
    src = np.ascontiguousarray(triplets[:, 0]).astype(np.int64)
    e_total = src.shape[0]
    epc = e_total // N_CORES
    n = ent_embed.shape[0]
    emb_pad = np.zeros((NP_TOTAL, F), np.float32)
    emb_pad[:n] = np.asarray(ent_embed, np.float32)
    wa = np.asarray(W_a, np.float32)
    ba = np.asarray(b_a, np.float32)
    wab = np.ascontiguousarray(
        np.concatenate([wa.T, ba[None, :]], axis=0))
    maps = []
    for c in range(N_CORES):
        s = src[c * epc:(c + 1) * epc]
        sp = np.empty(EPC, np.int64)
        sp[:epc] = s
        sp[epc:] = s[0]  # duplicate edge: idempotent for the membership mask
        emb_c = emb_pad[c * NPC:(c + 1) * NPC]
        maps.append({
            "lo": (sp // HI).astype(np.float32).reshape(128, B),
            "hi": (sp % HI).astype(np.float32).reshape(128, B),
            "embt": np.ascontiguousarray(emb_c.T),
            "emb": emb_c,
            "wab": wab,
        })
    return maps


def kernel(triplets, ent_embed, W_a, b_a, W_a2, b_a2):
    # W_a2 / b_a2 cancel algebraically (see module docstring)
    nc = _get_nc()
    maps = _in_maps(triplets, ent_embed, W_a, b_a)
    res = run_bass_kernel_spmd(nc, maps, core_ids=list(range(N_CORES)))
    out = np.concatenate([r["out"] for r in res.results], axis=0)
    return np.ascontiguousarray(out[:ent_embed.shape[0]])
